# revision 45
# baseline (speedup 1.0000x reference)
"""Embedding lookup + RMSNorm + tied logits projection on 8 trn2 NeuronCores.

Strategy (vocab-tensor-parallel, software-pipelined):
  - Pad vocab 50257 -> 51200 = 8 * 6400. Core c owns vocab rows [c*6400, (c+1)*6400).
  - final_norm is folded into the weight shard on the host; embedding and
    weights ship as bf16.
  - Phase 1 (replicated on every core): gather h = emb[idx] via indirect DMA
    (128 rows/instr), RMSNorm scale on-chip, transpose to hnT [d, t] bf16.
  - Phase 2: logits[s, v, t512] = sum_d WT[d, v] * hnT[d, t]; WT resident in
    SBUF (preloaded in 8 DRAM-contiguous pieces), lhsT = WT 128-col tile
    (stationary), rhs = hnT slice (moving, 512 tokens), f32 PSUM accumulation
    over 6 k-chunks, drained to bf16 (alternating DVE/Act engines).
  - Output is chunk-major [T/1024, VS, 1024] bf16 so every output DMA writes
    one fully contiguous 256KB DRAM block (the [VS, T] layout's 2KB-strided
    row writes measured ~4x slower on hardware), 200 DMAs total, alternating
    the SP/Act issue queues.
  - Tokens go in 4 chunks of 1024: the matmul sweep over chunk c overlaps
    phase-1 of chunk c+1, so only chunk 0's phase 1 is exposed; dummy
    identity transposes warm the PE clock during that head.
  - Host assembles: concat shards over vocab, upcast to f32, slice to 50257.
"""
import sys

sys.path.insert(0, "/opt/trn_rl_repo")

import numpy as np
import ml_dtypes

import concourse.bass as bass
import concourse.mybir as mybir
import concourse.tile as tile
from concourse import bacc
from concourse.bass import IndirectOffsetOnAxis
from concourse.bass_utils import run_bass_kernel_spmd
from concourse.masks import make_identity

f32 = mybir.dt.float32
bf16 = mybir.dt.bfloat16
i32 = mybir.dt.int32

B, S, V, D = 2, 2048, 50257, 768
T = B * S                 # 4096 tokens
NC = 8                    # cores
VS = 6400                 # vocab shard per core (51200 padded)
KK = D // 128             # 6 k-chunks
NVO = VS // 256           # 25 vo iterations (2 v-tiles each)
NGT = T // 128            # 32 gather tiles total
CHUNKS = [1024, 1024, 1024, 1024]
EPS = 1e-5

_cache = {}

OUT_NAME = "logitsT"


def _extract_shard(out, core):
    # out: [T//1024, VS, 1024] bf16 -> [T, VS] f32 shard of logits
    return out.transpose(1, 0, 2).reshape(VS, T).T.astype(np.float32)


def _build():
    nc = bacc.Bacc("TRN2", target_bir_lowering=False, debug=False, num_devices=NC)
    emb = nc.dram_tensor("emb", [V, D], bf16, kind="ExternalInput")
    idx = nc.dram_tensor("idx", [128, NGT], i32, kind="ExternalInput")
    wt = nc.dram_tensor("wt", [128, 8, KK, VS // 8], bf16, kind="ExternalInput")
    outT = nc.dram_tensor(OUT_NAME, [T // 1024, VS, 1024], bf16, kind="ExternalOutput")

    starts = np.cumsum([0] + CHUNKS[:-1]).tolist()

    with tile.TileContext(nc) as tc:
        with (
            tc.tile_pool(name="const", bufs=1) as constp,
            tc.tile_pool(name="wts", bufs=1) as wtsp,
            tc.tile_pool(name="hntp", bufs=1) as hntp,
            tc.tile_pool(name="gp", bufs=4) as gp,
            tc.tile_pool(name="sp", bufs=4) as sp,
            tc.tile_pool(name="outp", bufs=8) as outp,
            tc.tile_pool(name="tps", bufs=4, space="PSUM") as tps,
            tc.tile_pool(name="mpp", bufs=4, space="PSUM") as mpp,
        ):
            identb = constp.tile([128, 128], bf16)
            make_identity(nc, identb[:])
            epsc = constp.tile([128, 1], f32)
            nc.vector.memset(epsc[:], EPS)
            itall = constp.tile([128, NGT], i32)
            ng0 = CHUNKS[0] // 128
            nc.sync.dma_start(out=itall[:, :ng0], in_=idx[:, :ng0])

            hnt = hntp.tile([128, KK, T], bf16)
            wtsb = wtsp.tile([128, KK, VS], bf16)
            WQ = VS // 8

            def wt_piece(q):
                nc.sync.dma_start(out=wtsb[:, :, q * WQ:(q + 1) * WQ],
                                  in_=wt[:, q, :, :])

            for q in range(4):
                wt_piece(q)
            nc.scalar.dma_start(out=itall[:, ng0:], in_=idx[:, ng0:])

            # warm up the PE clock during the phase-1 head
            wps = tps.tile([128, 128], bf16, tag="tp", name="warm")
            for _ in range(72):
                nc.tensor.transpose(out=wps[:], in_=identb[:], identity=identb[:])

            def phase1_dma(gg):
                # gather DMA for global g-tile gg
                h = gp.tile([128, D], bf16, tag="h", name=f"h_{gg}")
                nc.gpsimd.indirect_dma_start(
                    out=h[:], out_offset=None, in_=emb[:],
                    in_offset=IndirectOffsetOnAxis(ap=itall[:, gg:gg + 1], axis=0),
                )
                return h

            def phase1_compute(gg, h):
                # rmsnorm scale + transpose into hnt for global g-tile gg;
                # DVE and GpSimd alternate so neither serializes the head
                eng = nc.vector if gg % 2 == 0 else nc.gpsimd
                sq = sp.tile([128, D], f32, tag="sq")
                nc.gpsimd.tensor_tensor(out=sq[:], in0=h[:], in1=h[:],
                                        op=mybir.AluOpType.mult)
                ssq = sp.tile([128, 1], f32, tag="ssq")
                nc.vector.tensor_reduce(out=ssq[:], in_=sq[:],
                                        axis=mybir.AxisListType.X,
                                        op=mybir.AluOpType.add)
                rms = sp.tile([128, 1], f32, tag="rms")
                nc.scalar.activation(out=rms[:], in_=ssq[:],
                                     func=mybir.ActivationFunctionType.Sqrt,
                                     bias=epsc[:, :1], scale=1.0 / D)
                rs = sp.tile([128, 1], f32, tag="rs")
                nc.vector.reciprocal(out=rs[:], in_=rms[:])
                hs = gp.tile([128, D], bf16, tag="hs")
                eng.tensor_scalar_mul(out=hs[:], in0=h[:], scalar1=rs[:, :1])
                for kk in range(KK):
                    pt = tps.tile([128, 128], bf16, tag="tp")
                    nc.tensor.transpose(out=pt[:], in_=hs[:, kk * 128:(kk + 1) * 128],
                                        identity=identb[:])
                    dst = hnt[:, kk, gg * 128:(gg + 1) * 128]
                    if kk % 2 == 0:
                        nc.scalar.copy(out=dst, in_=pt[:])
                    else:
                        nc.vector.tensor_copy(out=dst, in_=pt[:])

            # chunk-0 phase 1 upfront, emission interleaved so the gather
            # issue slices don't delay the first tiles' compute
            for g in range(ng0):
                h = phase1_dma(g)
                phase1_compute(g, h)

            for c, (t0, ct) in enumerate(zip(starts, CHUNKS)):
                # moving-operand slices of up to 512 tokens
                sl = [(o, min(512, ct - o)) for o in range(0, ct, 512)]
                last_chunk = c + 1 >= len(CHUNKS)
                if not last_chunk:
                    ng_next = CHUNKS[c + 1] // 128
                    gg_next0 = starts[c + 1] // 128
                pend = {}
                for vo in range(NVO):
                    if c == 0 and vo in (4, 8, 12, 16):
                        wt_piece(4 + (vo - 4) // 4)
                    if not last_chunk and vo < ng_next:
                        pend[vo] = phase1_dma(gg_next0 + vo)
                    for vi in range(2):
                        v = vo * 2 + vi
                        pss = [mpp.tile([128, w], f32, tag="mm",
                                        name=f"mm_{c}_{v}_{t}")
                               for t, (o, w) in enumerate(sl)]
                        for kk in range(KK):
                            for t, (o, w) in enumerate(sl):
                                nc.tensor.matmul(
                                    out=pss[t][:],
                                    lhsT=wtsb[:, kk, v * 128:(v + 1) * 128],
                                    rhs=hnt[:, kk, t0 + o:t0 + o + w],
                                    start=(kk == 0), stop=(kk == KK - 1),
                                )
                        final_vi = last_chunk and vo == NVO - 1 and vi == 1
                        if final_vi:
                            # shortest tail: per-slice DMAs; the very last
                            # slice splits across both engines and queues
                            for t, (o, w) in enumerate(sl):
                                if t < len(sl) - 1:
                                    obf = outp.tile([128, w], bf16,
                                                    name=f"obf_{t}")
                                    nc.vector.tensor_copy(out=obf[:],
                                                          in_=pss[t][:])
                                    nc.sync.dma_start(
                                        out=outT[c, v * 128:(v + 1) * 128,
                                                 o:o + w],
                                        in_=obf[:])
                                else:
                                    m = w // 2
                                    oba = outp.tile([128, m], bf16, name="oba")
                                    obb = outp.tile([128, w - m], bf16,
                                                    name="obb")
                                    nc.vector.tensor_copy(out=oba[:],
                                                          in_=pss[t][:, 0:m])
                                    nc.scalar.copy(out=obb[:],
                                                   in_=pss[t][:, m:w])
                                    nc.sync.dma_start(
                                        out=outT[c, v * 128:(v + 1) * 128,
                                                 o:o + m],
                                        in_=oba[:])
                                    nc.scalar.dma_start(
                                        out=outT[c, v * 128:(v + 1) * 128,
                                                 o + m:o + w],
                                        in_=obb[:])
                            continue
                        ob = outp.tile([128, ct], bf16, tag="ob",
                                       name=f"ob_{c}_{v}")
                        for t, (o, w) in enumerate(sl):
                            if (t + vi) % 2 == 0:
                                nc.vector.tensor_copy(out=ob[:, o:o + w],
                                                      in_=pss[t][:])
                            else:
                                nc.scalar.copy(out=ob[:, o:o + w], in_=pss[t][:])
                        # one contiguous 256KB DMA per (chunk, v-tile);
                        # alternate queues so issues don't serialize
                        dma_eng = nc.sync if vi == 0 else nc.scalar
                        dma_eng.dma_start(
                            out=outT[c, v * 128:(v + 1) * 128, :], in_=ob[:])
                    if not last_chunk and 5 <= vo < ng_next + 5:
                        g = vo - 5
                        phase1_compute(gg_next0 + g, pend.pop(g))

    nc.compile()
    return nc


def _in_maps(input_sequence, embedding, final_norm, output_embedding):
    idx_flat = np.asarray(input_sequence).astype(np.int32).reshape(-1)
    # itall[p, g] = idx[g*128 + p]
    idx_np = np.ascontiguousarray(idx_flat.reshape(NGT, 128).T)
    emb_np = np.ascontiguousarray(
        np.asarray(embedding, dtype=np.float32)).astype(ml_dtypes.bfloat16)
    fn = np.asarray(final_norm, dtype=np.float32)
    w = np.asarray(output_embedding, dtype=np.float32) * fn[None, :]
    w_pad = np.zeros((NC * VS, D), dtype=np.float32)
    w_pad[:V] = w
    maps = []
    for c in range(NC):
        wc = w_pad[c * VS:(c + 1) * VS]                       # [VS, D]
        wtc = np.ascontiguousarray(
            wc.T.reshape(KK, 128, VS).transpose(1, 0, 2)  # [128, KK, VS]
            .reshape(128, KK, 8, VS // 8).transpose(0, 2, 1, 3)  # piece-major
        ).astype(ml_dtypes.bfloat16)
        maps.append({"emb": emb_np, "idx": idx_np, "wt": wtc})
    return maps


def _run(in_maps, trace=False):
    if "nc" not in _cache:
        _cache["nc"] = _build()
    return run_bass_kernel_spmd(_cache["nc"], in_maps, list(range(NC)), trace=trace)


def kernel(input_sequence, embedding, final_norm, output_embedding):
    maps = _in_maps(input_sequence, embedding, final_norm, output_embedding)
    res = _run(maps)
    full = np.empty((T, NC * VS), dtype=np.float32)
    for c in range(NC):
        full[:, c * VS:(c + 1) * VS] = _extract_shard(res.results[c][OUT_NAME], c)
    return np.ascontiguousarray(full[:, :V]).reshape(B, S, V)


# revision 49
# speedup vs baseline: 1.0025x; 1.0025x over previous
"""Embedding lookup + RMSNorm + tied logits projection on 8 trn2 NeuronCores.

Strategy (vocab-tensor-parallel, software-pipelined):
  - Pad vocab 50257 -> 51200 = 8 * 6400. Core c owns vocab rows [c*6400, (c+1)*6400).
  - final_norm is folded into the weight shard on the host; embedding and
    weights ship as bf16.
  - Phase 1 (replicated on every core): gather h = emb[idx] via indirect DMA
    (128 rows/instr), RMSNorm scale on-chip, transpose to hnT [d, t] bf16.
  - Phase 2: logits[s, v, t512] = sum_d WT[d, v] * hnT[d, t]; WT resident in
    SBUF (preloaded in 8 DRAM-contiguous pieces), lhsT = WT 128-col tile
    (stationary), rhs = hnT slice (moving, 512 tokens), f32 PSUM accumulation
    over 6 k-chunks, drained to bf16 (alternating DVE/Act engines).
  - Output is chunk-major [T/1024, VS, 1024] bf16 so every output DMA writes
    one fully contiguous 256KB DRAM block (the [VS, T] layout's 2KB-strided
    row writes measured ~4x slower on hardware), 200 DMAs total, alternating
    the SP/Act issue queues.
  - Tokens go in 4 chunks of 1024: the matmul sweep over chunk c overlaps
    phase-1 of chunk c+1, so only chunk 0's phase 1 is exposed; dummy
    identity transposes warm the PE clock during that head.
  - Host assembles: concat shards over vocab, upcast to f32, slice to 50257.
"""
import sys

sys.path.insert(0, "/opt/trn_rl_repo")

import numpy as np
import ml_dtypes

import concourse.bass as bass
import concourse.mybir as mybir
import concourse.tile as tile
from concourse import bacc
from concourse.bass import IndirectOffsetOnAxis
from concourse.bass_utils import run_bass_kernel_spmd
from concourse.masks import make_identity

f32 = mybir.dt.float32
bf16 = mybir.dt.bfloat16
i32 = mybir.dt.int32

B, S, V, D = 2, 2048, 50257, 768
T = B * S                 # 4096 tokens
NC = 8                    # cores
VS = 6400                 # vocab shard per core (51200 padded)
KK = D // 128             # 6 k-chunks
NVO = VS // 256           # 25 vo iterations (2 v-tiles each)
NGT = T // 128            # 32 gather tiles total
CHUNKS = [1024, 1024, 1024, 1024]
EPS = 1e-5

_cache = {}

OUT_NAME = "logitsT"


def _extract_shard(out, core):
    # out: [T//1024, VS, 1024] bf16 -> [T, VS] f32 shard of logits
    return out.transpose(1, 0, 2).reshape(VS, T).T.astype(np.float32)


def _build():
    nc = bacc.Bacc("TRN2", target_bir_lowering=False, debug=False, num_devices=NC)
    emb = nc.dram_tensor("emb", [V, D], bf16, kind="ExternalInput")
    idx = nc.dram_tensor("idx", [128, NGT], i32, kind="ExternalInput")
    wt = nc.dram_tensor("wt", [128, 8, KK, VS // 8], bf16, kind="ExternalInput")
    outT = nc.dram_tensor(OUT_NAME, [T // 1024, VS, 1024], bf16, kind="ExternalOutput")

    starts = np.cumsum([0] + CHUNKS[:-1]).tolist()

    with tile.TileContext(nc) as tc:
        with (
            tc.tile_pool(name="const", bufs=1) as constp,
            tc.tile_pool(name="wts", bufs=1) as wtsp,
            tc.tile_pool(name="hntp", bufs=1) as hntp,
            tc.tile_pool(name="gp", bufs=4) as gp,
            tc.tile_pool(name="sp", bufs=4) as sp,
            tc.tile_pool(name="outp", bufs=8) as outp,
            tc.tile_pool(name="tps", bufs=4, space="PSUM") as tps,
            tc.tile_pool(name="mpp", bufs=4, space="PSUM") as mpp,
        ):
            identb = constp.tile([128, 128], bf16)
            make_identity(nc, identb[:])
            epsc = constp.tile([128, 1], f32)
            nc.vector.memset(epsc[:], EPS)
            itall = constp.tile([128, NGT], i32)
            ng0 = CHUNKS[0] // 128
            nc.sync.dma_start(out=itall[:, :ng0], in_=idx[:, :ng0])

            hnt = hntp.tile([128, KK, T], bf16)
            wtsb = wtsp.tile([128, KK, VS], bf16)
            WQ = VS // 8

            def wt_piece(q):
                nc.sync.dma_start(out=wtsb[:, :, q * WQ:(q + 1) * WQ],
                                  in_=wt[:, q, :, :])

            for q in range(4):
                wt_piece(q)
            nc.scalar.dma_start(out=itall[:, ng0:], in_=idx[:, ng0:])

            # warm up the PE clock during the phase-1 head
            wps = tps.tile([128, 128], bf16, tag="tp", name="warm")
            for _ in range(116):
                nc.tensor.transpose(out=wps[:], in_=identb[:], identity=identb[:])

            def phase1_dma(gg):
                # gather DMA for global g-tile gg
                h = gp.tile([128, D], bf16, tag="h", name=f"h_{gg}")
                nc.gpsimd.indirect_dma_start(
                    out=h[:], out_offset=None, in_=emb[:],
                    in_offset=IndirectOffsetOnAxis(ap=itall[:, gg:gg + 1], axis=0),
                )
                return h

            def phase1_compute(gg, h):
                # rmsnorm scale + transpose into hnt for global g-tile gg;
                # DVE and GpSimd alternate so neither serializes the head
                eng = nc.vector if gg % 2 == 0 else nc.gpsimd
                sq = sp.tile([128, D], f32, tag="sq")
                nc.gpsimd.tensor_tensor(out=sq[:], in0=h[:], in1=h[:],
                                        op=mybir.AluOpType.mult)
                ssq = sp.tile([128, 1], f32, tag="ssq")
                nc.vector.tensor_reduce(out=ssq[:], in_=sq[:],
                                        axis=mybir.AxisListType.X,
                                        op=mybir.AluOpType.add)
                rms = sp.tile([128, 1], f32, tag="rms")
                nc.scalar.activation(out=rms[:], in_=ssq[:],
                                     func=mybir.ActivationFunctionType.Sqrt,
                                     bias=epsc[:, :1], scale=1.0 / D)
                rs = sp.tile([128, 1], f32, tag="rs")
                nc.vector.reciprocal(out=rs[:], in_=rms[:])
                hs = gp.tile([128, D], bf16, tag="hs")
                eng.tensor_scalar_mul(out=hs[:], in0=h[:], scalar1=rs[:, :1])
                for kk in range(KK):
                    pt = tps.tile([128, 128], bf16, tag="tp")
                    nc.tensor.transpose(out=pt[:], in_=hs[:, kk * 128:(kk + 1) * 128],
                                        identity=identb[:])
                    dst = hnt[:, kk, gg * 128:(gg + 1) * 128]
                    if kk % 2 == 0:
                        nc.scalar.copy(out=dst, in_=pt[:])
                    else:
                        nc.vector.tensor_copy(out=dst, in_=pt[:])

            # chunk-0 phase 1 upfront, emission interleaved so the gather
            # issue slices don't delay the first tiles' compute
            for g in range(ng0):
                h = phase1_dma(g)
                phase1_compute(g, h)

            for c, (t0, ct) in enumerate(zip(starts, CHUNKS)):
                # moving-operand slices of up to 512 tokens
                sl = [(o, min(512, ct - o)) for o in range(0, ct, 512)]
                last_chunk = c + 1 >= len(CHUNKS)
                if not last_chunk:
                    ng_next = CHUNKS[c + 1] // 128
                    gg_next0 = starts[c + 1] // 128
                pend = {}
                for vo in range(NVO):
                    if c == 0 and vo in (4, 8, 12, 16):
                        wt_piece(4 + (vo - 4) // 4)
                    if not last_chunk and vo < ng_next:
                        pend[vo] = phase1_dma(gg_next0 + vo)
                    for vi in range(2):
                        v = vo * 2 + vi
                        pss = [mpp.tile([128, w], f32, tag="mm",
                                        name=f"mm_{c}_{v}_{t}")
                               for t, (o, w) in enumerate(sl)]
                        for kk in range(KK):
                            for t, (o, w) in enumerate(sl):
                                nc.tensor.matmul(
                                    out=pss[t][:],
                                    lhsT=wtsb[:, kk, v * 128:(v + 1) * 128],
                                    rhs=hnt[:, kk, t0 + o:t0 + o + w],
                                    start=(kk == 0), stop=(kk == KK - 1),
                                )
                        final_vi = last_chunk and vo == NVO - 1 and vi == 1
                        if final_vi:
                            # shortest tail: per-slice DMAs; the very last
                            # slice splits across both engines and queues
                            for t, (o, w) in enumerate(sl):
                                if t < len(sl) - 1:
                                    obf = outp.tile([128, w], bf16,
                                                    name=f"obf_{t}")
                                    nc.vector.tensor_copy(out=obf[:],
                                                          in_=pss[t][:])
                                    nc.sync.dma_start(
                                        out=outT[c, v * 128:(v + 1) * 128,
                                                 o:o + w],
                                        in_=obf[:])
                                else:
                                    m = w // 2
                                    oba = outp.tile([128, m], bf16, name="oba")
                                    obb = outp.tile([128, w - m], bf16,
                                                    name="obb")
                                    nc.vector.tensor_copy(out=oba[:],
                                                          in_=pss[t][:, 0:m])
                                    nc.scalar.copy(out=obb[:],
                                                   in_=pss[t][:, m:w])
                                    nc.sync.dma_start(
                                        out=outT[c, v * 128:(v + 1) * 128,
                                                 o:o + m],
                                        in_=oba[:])
                                    nc.scalar.dma_start(
                                        out=outT[c, v * 128:(v + 1) * 128,
                                                 o + m:o + w],
                                        in_=obb[:])
                            continue
                        ob = outp.tile([128, ct], bf16, tag="ob",
                                       name=f"ob_{c}_{v}")
                        for t, (o, w) in enumerate(sl):
                            if (t + vi) % 2 == 0:
                                nc.vector.tensor_copy(out=ob[:, o:o + w],
                                                      in_=pss[t][:])
                            else:
                                nc.scalar.copy(out=ob[:, o:o + w], in_=pss[t][:])
                        # one contiguous 256KB DMA per (chunk, v-tile);
                        # alternate queues so issues don't serialize
                        dma_eng = nc.sync if vi == 0 else nc.scalar
                        dma_eng.dma_start(
                            out=outT[c, v * 128:(v + 1) * 128, :], in_=ob[:])
                    if not last_chunk and 5 <= vo < ng_next + 5:
                        g = vo - 5
                        phase1_compute(gg_next0 + g, pend.pop(g))

    nc.compile()
    return nc


def _in_maps(input_sequence, embedding, final_norm, output_embedding):
    idx_flat = np.asarray(input_sequence).astype(np.int32).reshape(-1)
    # itall[p, g] = idx[g*128 + p]
    idx_np = np.ascontiguousarray(idx_flat.reshape(NGT, 128).T)
    emb_np = np.ascontiguousarray(
        np.asarray(embedding, dtype=np.float32)).astype(ml_dtypes.bfloat16)
    fn = np.asarray(final_norm, dtype=np.float32)
    w = np.asarray(output_embedding, dtype=np.float32) * fn[None, :]
    w_pad = np.zeros((NC * VS, D), dtype=np.float32)
    w_pad[:V] = w
    maps = []
    for c in range(NC):
        wc = w_pad[c * VS:(c + 1) * VS]                       # [VS, D]
        wtc = np.ascontiguousarray(
            wc.T.reshape(KK, 128, VS).transpose(1, 0, 2)  # [128, KK, VS]
            .reshape(128, KK, 8, VS // 8).transpose(0, 2, 1, 3)  # piece-major
        ).astype(ml_dtypes.bfloat16)
        maps.append({"emb": emb_np, "idx": idx_np, "wt": wtc})
    return maps


def _run(in_maps, trace=False):
    if "nc" not in _cache:
        _cache["nc"] = _build()
    return run_bass_kernel_spmd(_cache["nc"], in_maps, list(range(NC)), trace=trace)


def kernel(input_sequence, embedding, final_norm, output_embedding):
    maps = _in_maps(input_sequence, embedding, final_norm, output_embedding)
    res = _run(maps)
    full = np.empty((T, NC * VS), dtype=np.float32)
    for c in range(NC):
        full[:, c * VS:(c + 1) * VS] = _extract_shard(res.results[c][OUT_NAME], c)
    return np.ascontiguousarray(full[:, :V]).reshape(B, S, V)


# revision 50
# speedup vs baseline: 1.8043x; 1.7999x over previous
"""Embedding lookup + RMSNorm + tied logits projection on 8 trn2 NeuronCores.

Strategy (vocab-tensor-parallel, software-pipelined):
  - Pad vocab 50257 -> 51200 = 8 * 6400. Core c owns vocab rows [c*6400, (c+1)*6400).
  - final_norm is folded into the weight shard on the host; embedding and
    weights ship as bf16.
  - Phase 1 (replicated on every core): gather h = emb[idx] via indirect DMA
    (128 rows/instr), RMSNorm scale on-chip, transpose to hnT [d, t] bf16.
  - Phase 2: logits[s, v, t512] = sum_d WT[d, v] * hnT[d, t]; WT resident in
    SBUF (preloaded in 8 DRAM-contiguous pieces), lhsT = WT 128-col tile
    (stationary), rhs = hnT slice (moving, 512 tokens), f32 PSUM accumulation
    over 6 k-chunks, drained to bf16 (alternating DVE/Act engines).
  - Output is chunk-major [T/1024, VS, 1024] bf16 so every output DMA writes
    one fully contiguous 256KB DRAM block (the [VS, T] layout's 2KB-strided
    row writes measured ~4x slower on hardware), 200 DMAs total, alternating
    the SP/Act issue queues.
  - Tokens go in 4 chunks of 1024: the matmul sweep over chunk c overlaps
    phase-1 of chunk c+1, so only chunk 0's phase 1 is exposed; dummy
    identity transposes warm the PE clock during that head.
  - Host assembles: concat shards over vocab, upcast to f32, slice to 50257.
"""
import sys

sys.path.insert(0, "/opt/trn_rl_repo")

import numpy as np
import ml_dtypes

import concourse.bass as bass
import concourse.mybir as mybir
import concourse.tile as tile
from concourse import bacc
from concourse.bass import IndirectOffsetOnAxis
from concourse.bass_utils import run_bass_kernel_spmd
from concourse.masks import make_identity

f32 = mybir.dt.float32
bf16 = mybir.dt.bfloat16
i32 = mybir.dt.int32

B, S, V, D = 2, 2048, 50257, 768
T = B * S                 # 4096 tokens
NC = 8                    # cores
VS = 6400                 # vocab shard per core (51200 padded)
KK = D // 128             # 6 k-chunks
NVO = VS // 256           # 25 vo iterations (2 v-tiles each)
NGT = T // 128            # 32 gather tiles total
CHUNKS = [1024, 1024, 1024, 1024]
EPS = 1e-5

_cache = {}

OUT_NAME = "logitsT"


def _extract_shard(out, core):
    # out: [T//1024, VS, 1024] bf16 -> [T, VS] f32 shard of logits
    return out.transpose(1, 0, 2).reshape(VS, T).T.astype(np.float32)


def _build():
    nc = bacc.Bacc("TRN2", target_bir_lowering=False, debug=False, num_devices=NC)
    emb = nc.dram_tensor("emb", [V, D], bf16, kind="ExternalInput")
    idx = nc.dram_tensor("idx", [128, NGT], i32, kind="ExternalInput")
    wt = nc.dram_tensor("wt", [128, 8, KK, VS // 8], bf16, kind="ExternalInput")
    outT = nc.dram_tensor(OUT_NAME, [T // 1024, VS, 1024], bf16, kind="ExternalOutput")

    starts = np.cumsum([0] + CHUNKS[:-1]).tolist()

    with tile.TileContext(nc) as tc:
        with (
            tc.tile_pool(name="const", bufs=1) as constp,
            tc.tile_pool(name="wts", bufs=1) as wtsp,
            tc.tile_pool(name="hntp", bufs=1) as hntp,
            tc.tile_pool(name="gp", bufs=4) as gp,
            tc.tile_pool(name="sp", bufs=4) as sp,
            tc.tile_pool(name="outp", bufs=8) as outp,
            tc.tile_pool(name="tps", bufs=4, space="PSUM") as tps,
            tc.tile_pool(name="mpp", bufs=4, space="PSUM") as mpp,
        ):
            identb = constp.tile([128, 128], bf16)
            make_identity(nc, identb[:])
            epsc = constp.tile([128, 1], f32)
            nc.vector.memset(epsc[:], EPS)
            itall = constp.tile([128, NGT], i32)
            ng0 = CHUNKS[0] // 128
            nc.sync.dma_start(out=itall[:, :ng0], in_=idx[:, :ng0])

            hnt = hntp.tile([128, KK, T], bf16)
            wtsb = wtsp.tile([128, KK, VS], bf16)
            WQ = VS // 8

            def wt_piece(q):
                nc.sync.dma_start(out=wtsb[:, :, q * WQ:(q + 1) * WQ],
                                  in_=wt[:, q, :, :])

            for q in range(4):
                wt_piece(q)
            nc.scalar.dma_start(out=itall[:, ng0:], in_=idx[:, ng0:])

            # warm up the PE clock during the phase-1 head
            wps = tps.tile([128, 128], bf16, tag="tp", name="warm")
            for _ in range(116):
                nc.tensor.transpose(out=wps[:], in_=identb[:], identity=identb[:])

            def phase1_dma(gg):
                # gather DMA for global g-tile gg
                h = gp.tile([128, D], bf16, tag="h", name=f"h_{gg}")
                nc.gpsimd.indirect_dma_start(
                    out=h[:], out_offset=None, in_=emb[:],
                    in_offset=IndirectOffsetOnAxis(ap=itall[:, gg:gg + 1], axis=0),
                )
                return h

            def phase1_compute(gg, h):
                # rmsnorm scale + transpose into hnt for global g-tile gg;
                # DVE and GpSimd alternate so neither serializes the head
                eng = nc.vector if gg % 2 == 0 else nc.gpsimd
                sq = sp.tile([128, D], f32, tag="sq")
                nc.gpsimd.tensor_tensor(out=sq[:], in0=h[:], in1=h[:],
                                        op=mybir.AluOpType.mult)
                ssq = sp.tile([128, 1], f32, tag="ssq")
                nc.vector.tensor_reduce(out=ssq[:], in_=sq[:],
                                        axis=mybir.AxisListType.X,
                                        op=mybir.AluOpType.add)
                rms = sp.tile([128, 1], f32, tag="rms")
                nc.scalar.activation(out=rms[:], in_=ssq[:],
                                     func=mybir.ActivationFunctionType.Sqrt,
                                     bias=epsc[:, :1], scale=1.0 / D)
                rs = sp.tile([128, 1], f32, tag="rs")
                nc.vector.reciprocal(out=rs[:], in_=rms[:])
                hs = gp.tile([128, D], bf16, tag="hs")
                eng.tensor_scalar_mul(out=hs[:], in0=h[:], scalar1=rs[:, :1])
                for kk in range(KK):
                    pt = tps.tile([128, 128], bf16, tag="tp")
                    nc.tensor.transpose(out=pt[:], in_=hs[:, kk * 128:(kk + 1) * 128],
                                        identity=identb[:])
                    dst = hnt[:, kk, gg * 128:(gg + 1) * 128]
                    if kk % 2 == 0:
                        nc.scalar.copy(out=dst, in_=pt[:])
                    else:
                        nc.vector.tensor_copy(out=dst, in_=pt[:])

            # chunk-0 phase 1 upfront, emission interleaved so the gather
            # issue slices don't delay the first tiles' compute. After the
            # first 4 g-tiles (tokens 0..511) are transposed, run a few
            # slice-0-only matmul blocks so the PE isn't idle while tiles
            # 4..7 finish their gather/rms chain.
            K0 = 3
            early_ob = {}
            for g in range(ng0):
                h = phase1_dma(g)
                phase1_compute(g, h)
                if g == 3:
                    for vo_e in range(K0):
                        for vi_e in range(2):
                            v_e = vo_e * 2 + vi_e
                            ps_e = mpp.tile([128, 512], f32, tag="mm",
                                            name=f"mm_e_{v_e}")
                            for kk in range(KK):
                                nc.tensor.matmul(
                                    out=ps_e[:],
                                    lhsT=wtsb[:, kk, v_e * 128:(v_e + 1) * 128],
                                    rhs=hnt[:, kk, 0:512],
                                    start=(kk == 0), stop=(kk == KK - 1),
                                )
                            ob_e = outp.tile([128, CHUNKS[0]], bf16, tag="ob",
                                             name=f"ob_0_{v_e}")
                            if vi_e == 0:
                                nc.vector.tensor_copy(out=ob_e[:, 0:512],
                                                      in_=ps_e[:])
                            else:
                                nc.scalar.copy(out=ob_e[:, 0:512], in_=ps_e[:])
                            early_ob[v_e] = ob_e

            for c, (t0, ct) in enumerate(zip(starts, CHUNKS)):
                # moving-operand slices of up to 512 tokens
                sl = [(o, min(512, ct - o)) for o in range(0, ct, 512)]
                last_chunk = c + 1 >= len(CHUNKS)
                if not last_chunk:
                    ng_next = CHUNKS[c + 1] // 128
                    gg_next0 = starts[c + 1] // 128
                pend = {}
                for vo in range(NVO):
                    if c == 0 and vo in (4, 8, 12, 16):
                        wt_piece(4 + (vo - 4) // 4)
                    if not last_chunk and vo < ng_next:
                        pend[vo] = phase1_dma(gg_next0 + vo)
                    for vi in range(2):
                        v = vo * 2 + vi
                        early = c == 0 and v in early_ob
                        pss = [None if (early and t == 0) else
                               mpp.tile([128, w], f32, tag="mm",
                                        name=f"mm_{c}_{v}_{t}")
                               for t, (o, w) in enumerate(sl)]
                        for kk in range(KK):
                            for t, (o, w) in enumerate(sl):
                                if pss[t] is None:
                                    continue
                                nc.tensor.matmul(
                                    out=pss[t][:],
                                    lhsT=wtsb[:, kk, v * 128:(v + 1) * 128],
                                    rhs=hnt[:, kk, t0 + o:t0 + o + w],
                                    start=(kk == 0), stop=(kk == KK - 1),
                                )
                        final_vi = last_chunk and vo == NVO - 1 and vi == 1
                        if final_vi:
                            # shortest tail: per-slice DMAs; the very last
                            # slice splits across both engines and queues
                            for t, (o, w) in enumerate(sl):
                                if t < len(sl) - 1:
                                    obf = outp.tile([128, w], bf16,
                                                    name=f"obf_{t}")
                                    nc.vector.tensor_copy(out=obf[:],
                                                          in_=pss[t][:])
                                    nc.sync.dma_start(
                                        out=outT[c, v * 128:(v + 1) * 128,
                                                 o:o + w],
                                        in_=obf[:])
                                else:
                                    m = w // 2
                                    oba = outp.tile([128, m], bf16, name="oba")
                                    obb = outp.tile([128, w - m], bf16,
                                                    name="obb")
                                    nc.vector.tensor_copy(out=oba[:],
                                                          in_=pss[t][:, 0:m])
                                    nc.scalar.copy(out=obb[:],
                                                   in_=pss[t][:, m:w])
                                    nc.sync.dma_start(
                                        out=outT[c, v * 128:(v + 1) * 128,
                                                 o:o + m],
                                        in_=oba[:])
                                    nc.scalar.dma_start(
                                        out=outT[c, v * 128:(v + 1) * 128,
                                                 o + m:o + w],
                                        in_=obb[:])
                            continue
                        if c == 0 and v in early_ob:
                            ob = early_ob.pop(v)
                        else:
                            ob = outp.tile([128, ct], bf16, tag="ob",
                                           name=f"ob_{c}_{v}")
                        for t, (o, w) in enumerate(sl):
                            if pss[t] is None:
                                continue
                            if (t + vi) % 2 == 0:
                                nc.vector.tensor_copy(out=ob[:, o:o + w],
                                                      in_=pss[t][:])
                            else:
                                nc.scalar.copy(out=ob[:, o:o + w], in_=pss[t][:])
                        # one contiguous 256KB DMA per (chunk, v-tile);
                        # alternate queues so issues don't serialize
                        dma_eng = nc.sync if vi == 0 else nc.scalar
                        dma_eng.dma_start(
                            out=outT[c, v * 128:(v + 1) * 128, :], in_=ob[:])
                    if not last_chunk and 5 <= vo < ng_next + 5:
                        g = vo - 5
                        phase1_compute(gg_next0 + g, pend.pop(g))

    nc.compile()
    return nc


def _in_maps(input_sequence, embedding, final_norm, output_embedding):
    idx_flat = np.asarray(input_sequence).astype(np.int32).reshape(-1)
    # itall[p, g] = idx[g*128 + p]
    idx_np = np.ascontiguousarray(idx_flat.reshape(NGT, 128).T)
    emb_np = np.ascontiguousarray(
        np.asarray(embedding, dtype=np.float32)).astype(ml_dtypes.bfloat16)
    fn = np.asarray(final_norm, dtype=np.float32)
    w = np.asarray(output_embedding, dtype=np.float32) * fn[None, :]
    w_pad = np.zeros((NC * VS, D), dtype=np.float32)
    w_pad[:V] = w
    maps = []
    for c in range(NC):
        wc = w_pad[c * VS:(c + 1) * VS]                       # [VS, D]
        wtc = np.ascontiguousarray(
            wc.T.reshape(KK, 128, VS).transpose(1, 0, 2)  # [128, KK, VS]
            .reshape(128, KK, 8, VS // 8).transpose(0, 2, 1, 3)  # piece-major
        ).astype(ml_dtypes.bfloat16)
        maps.append({"emb": emb_np, "idx": idx_np, "wt": wtc})
    return maps


def _run(in_maps, trace=False):
    if "nc" not in _cache:
        _cache["nc"] = _build()
    return run_bass_kernel_spmd(_cache["nc"], in_maps, list(range(NC)), trace=trace)


def kernel(input_sequence, embedding, final_norm, output_embedding):
    maps = _in_maps(input_sequence, embedding, final_norm, output_embedding)
    res = _run(maps)
    full = np.empty((T, NC * VS), dtype=np.float32)
    for c in range(NC):
        full[:, c * VS:(c + 1) * VS] = _extract_shard(res.results[c][OUT_NAME], c)
    return np.ascontiguousarray(full[:, :V]).reshape(B, S, V)
